# revision 1
# baseline (speedup 1.0000x reference)
"""MoC sparse attention (nn_MoCAttention) on 8 Trainium2 NeuronCores.

Strategy (head-parallel, hint-aligned): one attention head per core.
  - Host passes xT [512,2048] plus per-head weight slices to each core.
  - Routing (top-5 chunk selection) must match the fp32 reference's
    jax.lax.top_k exactly, so sims are computed with *exact fp32* matmuls
    via the associativity trick  sims = x @ (Wq_h @ (Wk_h^T @ xsum^T))
    which keeps every routing matmul tiny (chunk dim = 32).
  - Sparse block attention is done as masked dense attention: the top-5
    chunk mask is folded into the scores matmul for free by augmenting the
    contraction dim (KT gets 32 one-hot chunk-indicator rows, QT gets 32
    rows of -BIG*notmask), so exp() flushes non-selected chunks to 0.
  - Softmax denominator rides a ones-column appended to V; the divide is
    deferred until after the Wo matmul where it is a per-partition scalar.
  - Heavy matmuls run in float32r (11-bit mantissa, full PE rate).
  - Each core emits its head's partial output (out_h @ Wo[64h:64h+64,:]);
    the host sums the 8 partials.
"""
import sys

sys.path.insert(0, "/opt/trn_rl_repo")

import numpy as np

import concourse.bass as bass  # noqa: F401  (registers types)
import concourse.mybir as mybir
import concourse.tile as tile
from concourse import bacc, bass_utils
from concourse.masks import make_identity

H = 8
S = 2048
D = 512
HD = 64
CHUNK = 64
C = 32  # number of chunks
TOPK = 5
SCALE = HD ** -0.5
BIG = 1.0e4

NQB = 4          # query blocks of 512
QB = S // NQB    # 512
NST = 16         # query tiles of 128
NKT = 16         # key tiles of 128
NDT = 4          # d-chunks of 128

f32 = mybir.dt.float32
f32r = mybir.dt.float32r
f16 = mybir.dt.float16
AF = mybir.ActivationFunctionType
Alu = mybir.AluOpType


def _emit(nc, tc, xT_d, wqk_d, wv_d, wqT_d, wo_d, eoh_d, id32_d, id16_d,
          out_d,
          dbg_sims_d=None, dbg_nm_d=None, dbg_xsum_d=None, dbg_csk_d=None,
          dbg_m_d=None):
    ctx_pools = []

    def pool(name, bufs, space="SBUF"):
        p = tc.tile_pool(name=name, bufs=bufs, space=space)
        return p

    with (
        pool("persist", 1) as persist,
        pool("weights", 1) as weights,
    ):
        # ---- persistent SBUF tensors
        xts = []   # exact f32 copies (routing must be fp32-exact)
        xtrs = []  # f32r copies (fast-path matmul operands)
        for d in range(NDT):
            xts.append(persist.tile([128, S], f32, name=f"xt{d}", tag=f"xt{d}"))
            xtrs.append(persist.tile([128, S], f16, name=f"xtr{d}",
                                     tag=f"xtr{d}"))
        KM = persist.tile([96, S], f16, name="KM")
        QMs = [persist.tile([96, QB], f16, name=f"QM{qb}", tag=f"QM{qb}")
               for qb in range(NQB)]
        VT = persist.tile([HD, S], f32r, name="VT")
        V_aug = persist.tile([128, NKT, HD + 1], f16, name="V_aug")
        outTas = [persist.tile([HD + 1, QB], f16, name=f"outTa{qb}",
                               tag=f"outTa{qb}") for qb in range(NQB)]
        denomT = persist.tile([128, NST], f32, name="denomT")
        rdenomT = persist.tile([128, NST], f32, name="rdenomT")
        ident_f = persist.tile([128, 128], f32, name="ident_f")
        ident_r = persist.tile([128, 128], f32r, name="ident_r")
        ones_col = persist.tile([128, 1], f16, name="ones_col")
        ident_h = persist.tile([128, 128], f16, name="ident_h")
        cskT = persist.tile([HD, C], f32, name="cskT")
        Ms = [persist.tile([128, C], f32, name=f"M{d}", tag=f"M{d}")
              for d in range(NDT)]
        xlos = [persist.tile([128, S], f16, name=f"xlo{d}", tag=f"xlo{d}")
                for d in range(NDT)]
        M_his = [persist.tile([128, C], f16, name=f"Mhi{d}", tag=f"Mhi{d}")
                 for d in range(NDT)]
        M_los = [persist.tile([128, C], f16, name=f"Mlo{d}", tag=f"Mlo{d}")
                 for d in range(NDT)]

        wqk_f = [weights.tile([128, 2 * HD], f32, name=f"wqkf{d}",
                              tag=f"wqkf{d}") for d in range(NDT)]
        wqk_sb = [weights.tile([128, 2 * HD], f16, name=f"wqk{d}", tag=f"wqk{d}")
                  for d in range(NDT)]
        wv_sb = [weights.tile([128, HD], f16, name=f"wv{d}", tag=f"wv{d}")
                 for d in range(NDT)]
        wqT_sb = weights.tile([HD, D], f32, name="wqT_sb")
        wo_sb = weights.tile([HD, D], f16, name="wo_sb")

        # ---- identity + PE warm-up spin first (PE otherwise idles in DMA head)
        nc.sync.dma_start(out=ident_f, in_=id32_d)
        nc.sync.dma_start(out=ident_h, in_=id16_d)
        nc.sync.dma_start(out=ident_r, in_=id32_d.bitcast(f32r))
        with pool("ps_warm", 1, space="PSUM") as ps_warm:
            p_warm = ps_warm.tile([128, 128], f32, name="p_warm")
            for _ in range(40):
                nc.tensor.matmul(p_warm, ident_h, ident_h,
                                 start=True, stop=True)

        # ---- input DMAs
        for d in range(NDT):
            nc.sync.dma_start(out=wqk_f[d],
                              in_=wqk_d[d * 128:(d + 1) * 128, :])
            nc.vector.tensor_copy(out=wqk_sb[d], in_=wqk_f[d])
            nc.sync.dma_start(out=wv_sb[d],
                              in_=wv_d[d * 128:(d + 1) * 128, :])
        dma_engines = [nc.sync, nc.scalar, nc.gpsimd, nc.sync]
        for qb in range(NQB):
            for d in range(NDT):
                dma_engines[d].dma_start(
                    out=xts[d][:, qb * QB:(qb + 1) * QB],
                    in_=xT_d[d * 128:(d + 1) * 128, qb * QB:(qb + 1) * QB],
                )
                nc.vector.tensor_copy(
                    out=xtrs[d][:, qb * QB:(qb + 1) * QB],
                    in_=xts[d][:, qb * QB:(qb + 1) * QB],
                )
                nc.vector.tensor_sub(
                    out=xlos[d][:, qb * QB:(qb + 1) * QB],
                    in0=xts[d][:, qb * QB:(qb + 1) * QB],
                    in1=xtrs[d][:, qb * QB:(qb + 1) * QB],
                )
        nc.sync.dma_start(out=wqT_sb, in_=wqT_d)
        nc.sync.dma_start(out=wo_sb, in_=wo_d)

        nc.vector.memset(ones_col, 1.0)
        # V_aug ones column (softmax denominator rider); f32r memset is not
        # a legal ISA op, so memset f32 then copy (copy rounds to f32r)
        ones16 = persist.tile([128, NKT], f32, name="ones16")
        nc.vector.memset(ones16, 1.0)
        nc.vector.tensor_copy(out=V_aug[:, :, HD], in_=ones16)
        # KM rows 64..95 = one-hot chunk indicator E[c, k] = (k // 64 == c)
        nc.sync.dma_start(out=KM[HD:96, :], in_=eoh_d)

        # ---- projections: QK^T packed, VT
        with pool("ps_proj", 2, space="PSUM") as ps_proj:
            for qb in range(NQB):
                p_qk = ps_proj.tile([128, QB], f32, name="p_qk", tag="p_qk")
                for d in range(NDT):
                    nc.tensor.matmul(
                        p_qk, wqk_sb[d], xtrs[d][:, qb * QB:(qb + 1) * QB],
                        start=(d == 0), stop=(d == NDT - 1),
                    )
                nc.vector.tensor_copy(out=QMs[qb][0:HD, :], in_=p_qk[0:HD, :])
                nc.scalar.copy(out=KM[0:HD, qb * QB:(qb + 1) * QB],
                               in_=p_qk[HD:128, :])
            for qb in range(NQB):
                p_vt = ps_proj.tile([HD, QB], f32, name="p_vt", tag="p_vt")
                for d in range(NDT):
                    nc.tensor.matmul(
                        p_vt, wv_sb[d], xtrs[d][:, qb * QB:(qb + 1) * QB],
                        start=(d == 0), stop=(d == NDT - 1),
                    )
                nc.scalar.copy(out=VT[:, qb * QB:(qb + 1) * QB], in_=p_vt)

        # ---- V_aug via PE transposes of VT
        with pool("ps_vtr", 2, space="PSUM") as ps_vtr:
            for kt in range(NKT):
                p_v = ps_vtr.tile([128, HD], f32r, name="p_v", tag="p_v")
                nc.tensor.transpose(p_v, VT[:, kt * 128:(kt + 1) * 128],
                                    ident_r[0:HD, 0:HD])
                nc.scalar.copy(out=V_aug[:, kt, 0:HD], in_=p_v)

        # ---- routing: exact-fp32 sims = x @ (WqT^T @ (Wv... Wk path))
        # xsumT[d] [128, C] = per-chunk sums of x (exact adds on DVE)
        with pool("ps_route", 2, space="PSUM") as ps_route:
            xsumT = [persist.tile([128, C], f32, name=f"xsumT{d}", tag=f"xs{d}")
                     for d in range(NDT)]
            for d in range(NDT):
                nc.vector.reduce_sum(
                    out=xsumT[d],
                    in_=xts[d].rearrange("p (c k) -> p c k", k=CHUNK),
                    axis=mybir.AxisListType.X,
                )
            # cskT [64, 32] = sum_d Wk_d^T @ xsumT_d   (Wk = wqk cols 64:128)
            p_csk = ps_route.tile([HD, C], f32, name="p_csk", tag="p_small")
            for d in range(NDT):
                nc.tensor.matmul(
                    p_csk, wqk_f[d][:, HD:2 * HD], xsumT[d],
                    start=(d == 0), stop=(d == NDT - 1),
                )
            nc.vector.tensor_copy(out=cskT, in_=p_csk)
            if dbg_csk_d is not None:
                nc.sync.dma_start(out=dbg_csk_d, in_=cskT)
                for d in range(NDT):
                    nc.sync.dma_start(
                        out=dbg_xsum_d[d * 128:(d + 1) * 128, :], in_=xsumT[d])
            # M_d [128, 32] = WqT[:, dcols]^T @ cskT
            for d in range(NDT):
                p_m = ps_route.tile([128, C], f32, name="p_m", tag="p_small")
                nc.tensor.matmul(p_m, wqT_sb[:, d * 128:(d + 1) * 128], cskT,
                                 start=True, stop=True)
                nc.vector.tensor_copy(out=Ms[d], in_=p_m)
                nc.vector.tensor_copy(out=M_his[d], in_=Ms[d])
                nc.vector.tensor_sub(out=M_los[d], in0=Ms[d], in1=M_his[d])
                if dbg_m_d is not None:
                    nc.sync.dma_start(out=dbg_m_d[d * 128:(d + 1) * 128, :],
                                      in_=Ms[d])

        # routing: simsT = M^T @ x (exact fp32, stationary M amortized),
        # then per-s-tile PE transpose -> top8 -> notmask -> bias rows of QM
        simsT_sb = persist.tile([C, S], f32, name="simsT_sb")
        with (
            pool("ps_simsT", 2, space="PSUM") as ps_simsT,
            pool("ps_sims", 4, space="PSUM") as ps_sims,
            pool("ps_nmT", 2, space="PSUM") as ps_nmT,
            pool("rt_sb", 6) as rt_sb,
        ):
            for qb in range(NQB):
                p_simsT = ps_simsT.tile([C, QB], f32, name="p_simsT",
                                        tag="p_simsT")
                terms = []
                for d in range(NDT):
                    xhi = xtrs[d][:, qb * QB:(qb + 1) * QB]
                    xlo = xlos[d][:, qb * QB:(qb + 1) * QB]
                    terms += [(M_his[d], xhi), (M_los[d], xhi), (M_his[d], xlo)]
                for t, (lhsT, rhs) in enumerate(terms):
                    nc.tensor.matmul(p_simsT, lhsT, rhs,
                                     start=(t == 0), stop=(t == len(terms) - 1))
                nc.vector.tensor_copy(
                    out=simsT_sb[:, qb * QB:(qb + 1) * QB], in_=p_simsT)
                notmask4 = rt_sb.tile([128, NST // NQB, C], f32, name="notmask4",
                                      tag="nm4")
                for j in range(NST // NQB):
                    st = qb * (NST // NQB) + j
                    p_sims = ps_sims.tile([128, C], f32, name="p_sims",
                                          tag="p_sims")
                    nc.tensor.transpose(
                        p_sims, simsT_sb[:, st * 128:(st + 1) * 128],
                        ident_f[0:C, 0:C])
                    top8 = rt_sb.tile([128, 8], f32, name="top8", tag="top8")
                    nc.vector.max(out=top8, in_=p_sims)
                    nc.vector.tensor_scalar(
                        out=notmask4[:, j, :], in0=p_sims,
                        scalar1=top8[:, TOPK - 1:TOPK],
                        scalar2=None, op0=Alu.is_lt,
                    )
                    if dbg_sims_d is not None:
                        sims = rt_sb.tile([128, C], f32, name="sims", tag="sims")
                        nc.vector.tensor_copy(out=sims, in_=p_sims)
                        nc.sync.dma_start(
                            out=dbg_sims_d[st * 128:(st + 1) * 128, :], in_=sims)
                        nc.sync.dma_start(
                            out=dbg_nm_d[st * 128:(st + 1) * 128, :],
                            in_=notmask4[:, j, :])
                p_nmT = ps_nmT.tile([128, 128], f32, name="p_nmT", tag="p_nmT")
                nc.tensor.transpose(p_nmT, notmask4, ident_f)
                for j in range(NST // NQB):
                    nc.vector.tensor_scalar_mul(
                        out=QMs[qb][HD:96, j * 128:(j + 1) * 128],
                        in0=p_nmT[j * C:(j + 1) * C, :], scalar1=-BIG,
                    )

        # ---- main attention loop
        KG = 2   # k-tiles per score/exp group
        with (
            pool("ps_sc", 3, space="PSUM") as ps_sc,
            pool("ps_pv", 1, space="PSUM") as ps_pv,
            pool("ps_tail", 1, space="PSUM") as ps_tail,
            pool("exp_sb", 3) as exp_sb,
            pool("out_sb", 3) as out_sb_pool,
        ):
            for qb in range(NQB):
                p_pv = ps_pv.tile([HD + 1, QB], f32, name="p_pv", tag="p_pv")
                for g in range(NKT // KG):
                    p_sc = ps_sc.tile([128, KG * QB], f32, name="p_sc", tag="p_sc")
                    for i in range(KG):
                        kt = KG * g + i
                        nc.tensor.matmul(
                            p_sc[:, i * QB:(i + 1) * QB],
                            KM[:, kt * 128:(kt + 1) * 128],
                            QMs[qb],
                            start=True, stop=True,
                        )
                    expT = exp_sb.tile([128, KG * QB], f16, name="expT",
                                       tag="expT")
                    nc.scalar.activation(out=expT, in_=p_sc, func=AF.Exp,
                                         scale=SCALE)
                    for i in range(KG):
                        kt = KG * g + i
                        nc.tensor.matmul(
                            p_pv, V_aug[:, kt, :], expT[:, i * QB:(i + 1) * QB],
                            start=(kt == 0), stop=(kt == NKT - 1),
                        )
                nc.vector.tensor_copy(out=outTas[qb], in_=p_pv)
                # denominator row -> column(s) via K=1 ones matmul
                for j in range(4):
                    st = 4 * qb + j
                    p_dn = ps_tail.tile([128, 1], f32, name="p_dn", tag="p_tail")
                    nc.tensor.matmul(
                        p_dn, outTas[qb][HD:HD + 1, j * 128:(j + 1) * 128],
                        ones_col[HD:HD + 1, 0:1], start=True, stop=True,
                    )
                    nc.vector.tensor_copy(out=denomT[:, st:st + 1], in_=p_dn)
                nc.vector.reciprocal(out=rdenomT[:, 4 * qb:4 * qb + 4],
                                     in_=denomT[:, 4 * qb:4 * qb + 4])
                # Wo partial + normalize + store
                for j in range(4):
                    st = 4 * qb + j
                    p_wo = ps_tail.tile([128, D], f32, name="p_wo", tag="p_tail")
                    nc.tensor.matmul(
                        p_wo, outTas[qb][0:HD, j * 128:(j + 1) * 128], wo_sb,
                        start=True, stop=True,
                    )
                    o_sb = out_sb_pool.tile([128, D], f32, name="o_sb", tag="o_sb")
                    nc.vector.tensor_scalar(
                        out=o_sb, in0=p_wo, scalar1=rdenomT[:, st:st + 1],
                        scalar2=None, op0=Alu.mult,
                    )
                    nc.sync.dma_start(out=out_d[st * 128:(st + 1) * 128, :],
                                      in_=o_sb)


_CACHED_NC = None


def _build():
    global _CACHED_NC
    if _CACHED_NC is not None:
        return _CACHED_NC
    nc = bacc.Bacc("TRN2", target_bir_lowering=False, debug=False)
    xT_d = nc.dram_tensor("xT", [D, S], f32, kind="ExternalInput").ap()
    wqk_d = nc.dram_tensor("wqk", [D, 2 * HD], f32, kind="ExternalInput").ap()
    wv_d = nc.dram_tensor("wv", [D, HD], f16, kind="ExternalInput").ap()
    wqT_d = nc.dram_tensor("wqT", [HD, D], f32, kind="ExternalInput").ap()
    wo_d = nc.dram_tensor("wo", [HD, D], f16, kind="ExternalInput").ap()
    eoh_d = nc.dram_tensor("eoh", [C, S], f16, kind="ExternalInput").ap()
    id32_d = nc.dram_tensor("id32", [128, 128], f32, kind="ExternalInput").ap()
    id16_d = nc.dram_tensor("id16", [128, 128], f16, kind="ExternalInput").ap()
    import os
    dbg = os.environ.get("KERNEL_DEBUG") == "1"
    dbg_sims_d = dbg_nm_d = None
    dbg_xsum_d = dbg_csk_d = dbg_m_d = None
    if dbg:
        dbg_sims_d = nc.dram_tensor("dbg_sims", [S, C], f32,
                                    kind="ExternalOutput").ap()
        dbg_xsum_d = nc.dram_tensor("dbg_xsum", [D, C], f32,
                                    kind="ExternalOutput").ap()
        dbg_csk_d = nc.dram_tensor("dbg_csk", [HD, C], f32,
                                   kind="ExternalOutput").ap()
        dbg_m_d = nc.dram_tensor("dbg_m", [D, C], f32,
                                 kind="ExternalOutput").ap()
        dbg_nm_d = nc.dram_tensor("dbg_nm", [S, C], f32,
                                  kind="ExternalOutput").ap()
    out_d = nc.dram_tensor("out", [S, D], f32, kind="ExternalOutput").ap()
    with tile.TileContext(nc) as tc:
        _emit(nc, tc, xT_d, wqk_d, wv_d, wqT_d, wo_d, eoh_d, id32_d, id16_d,
              out_d, dbg_sims_d, dbg_nm_d, dbg_xsum_d, dbg_csk_d, dbg_m_d)
    nc.compile()
    _CACHED_NC = nc
    return nc


def _in_maps(x, Wq, Wk, Wv, Wo):
    x = np.ascontiguousarray(np.asarray(x, dtype=np.float32))
    Wq = np.asarray(Wq, dtype=np.float32)
    Wk = np.asarray(Wk, dtype=np.float32)
    Wv = np.asarray(Wv, dtype=np.float32)
    Wo = np.asarray(Wo, dtype=np.float32)
    xT = np.ascontiguousarray(x.reshape(S, D).T)
    eoh = np.kron(np.eye(C, dtype=np.float16), np.ones((1, CHUNK), np.float16))
    eoh = np.ascontiguousarray(eoh)
    ident32 = np.eye(128, dtype=np.float32)
    ident16 = np.eye(128, dtype=np.float16)
    maps = []
    for h in range(H):
        sl = slice(HD * h, HD * (h + 1))
        maps.append({
            "xT": xT,
            "wqk": np.ascontiguousarray(
                np.concatenate([Wq[:, sl], Wk[:, sl]], axis=1)),
            "wv": np.ascontiguousarray(Wv[:, sl]).astype(np.float16),
            "wqT": np.ascontiguousarray(Wq[:, sl].T),
            "wo": np.ascontiguousarray(Wo[sl, :]).astype(np.float16),
            "eoh": eoh,
            "id32": ident32,
            "id16": ident16,
        })
    return maps


def _ensure_profile_hook():
    """Register antenv.axon_hooks (NTFF profiling shim) if missing."""
    import importlib.util
    if importlib.util.find_spec("antenv.axon_hooks") is not None:
        return
    import importlib.machinery
    import antenv
    path = "/opt/trn_rl_repo/antenv/axon_hooks.py"
    loader = importlib.machinery.SourceFileLoader("antenv.axon_hooks", path)
    spec = importlib.util.spec_from_loader(loader.name, loader)
    mod = importlib.util.module_from_spec(spec)
    loader.exec_module(mod)
    sys.modules["antenv.axon_hooks"] = mod
    antenv.axon_hooks = mod


def run(x, Wq, Wk, Wv, Wo, trace=False):
    if trace:
        _ensure_profile_hook()
    nc = _build()
    res = bass_utils.run_bass_kernel_spmd(
        nc, _in_maps(x, Wq, Wk, Wv, Wo), core_ids=list(range(H)), trace=trace)
    acc = np.zeros((S, D), dtype=np.float64)
    for r in res.results:
        acc += r["out"].astype(np.float64)
    return acc.astype(np.float32).reshape(1, S, D), res


def kernel(x, Wq, Wk, Wv, Wo):
    out, _ = run(x, Wq, Wk, Wv, Wo)
    return out

